# revision 24
# baseline (speedup 1.0000x reference)
"""Trainium2 Bass kernel for LocalNodeAttentionHeadSum.

Computation (per batch b, pixel p=(h,w)):
    q[d,p]   = sum_c x[c,TMID,p] Wq[c,d] + bq[d]
    k[t,d]   = sum_c nodes[t,c] Wk[c,d] + bk[d]
    s[t,p]   = sum_d q[d,p] k[t,d];  alpha = softmax_t(s)
    y[d,p]   = sum_t alpha[t,p] * (sum_c x[c,t,p] Wv[c,d] + bv[d])
             = sum_c (sum_t alpha[t,p] x[c,t,p]) Wv[c,d] + bv[d]   (sum_t alpha = 1)
    out[c,p] = sum_d y[d,p] Wo[d,c] + bo[c]

Sharding: data-parallel over batch B=32 across 8 cores (4 batches/core).

Host-side weight folding (batch-independent, numpy): the query projection
and key path fold into  Wqk = Wq @ (nodes @ Wk + bk).T  ([C, T]) and
sb0[t] = bq . k[t], so the device never sees Wq/Wk/nodes/bq/bk; scores
are 8 thin [128c x 7] matmuls per batch.  Wv / Wo ship as fp16.

DMA regime (the kernel is HBM-bound): x streams through fp32->fp16
casting SWDGE DMAs (gpsimd), which move half the bytes into SBUF that an
fp32 load would.  Each batch is two DMAs - the t=0..3 frames (includes
the middle frame, unlocking scores/softmax early) and the t=4..6 frames -
with every descriptor a contiguous >=1 KB run; the x stream is issued
ahead of the weight loads so batch 0 lands as early as possible.  The
output is stored as fp16 with per-partition-contiguous 3 KB descriptors
and widened to fp32 on the host.

fp16 (not bf16) is deliberate: same DMA/DVE/PE cost in every model path,
but 10 mantissa bits keep the score error ~8x smaller, which the softmax
then cannot amplify into the output.

Engine split per batch: PE does scores + alpha row-broadcast + the final
5-way temporal accumulation + both projections; DVE does the alpha
multiplies and the first add level; ACT applies biases, exp and
PSUM->SBUF moves; GPSIMD does DMA descriptor gen and the per-pixel
partition max for softmax.
"""

import sys

for _p in ("/opt/trn_rl_repo",):
    if _p not in sys.path:
        sys.path.insert(0, _p)

from contextlib import ExitStack

import numpy as np

import concourse.bass as bass
import concourse.tile as tile
from concourse import bacc, mybir, bass_isa
from concourse.bass_utils import run_bass_kernel_spmd

F32 = mybir.dt.float32
F16 = mybir.dt.float16

# Problem shapes (hardcoded per contract)
B, C, T, H, W = 32, 1024, 7, 14, 14
D = 512
NCORES = 8
BL = B // NCORES          # 4 batches per core
HWF = H * W               # 196
THW = T * HWF             # 1372
CC = C // 128             # 8 chunks over input channels
DC = D // 128             # 4 chunks over inter channels
TMID = T // 2             # 3 (middle frame)
TA = 4                    # first t-group (t=0..3, includes TMID)
TB = T - TA               # second t-group (t=4..6)
FA = TA * HWF             # 784
FB = TB * HWF             # 588
PSLOT = 512               # full psum bank (f32 elems) per small tile

Exp = mybir.ActivationFunctionType.Exp
Identity = mybir.ActivationFunctionType.Identity


def build_program():
    nc = bacc.Bacc("TRN2", target_bir_lowering=False, debug=False)

    x_d = nc.dram_tensor("x_window", [BL, C, T, H, W], F32, kind="ExternalInput").ap()
    wqk_d = nc.dram_tensor("wqk", [128, CC * T], F16, kind="ExternalInput").ap()
    sb0_d = nc.dram_tensor("sb0", [T, 1], F32, kind="ExternalInput").ap()
    wv_d = nc.dram_tensor("Wv", [C, D], F16, kind="ExternalInput").ap()
    wo_d = nc.dram_tensor("Wo", [D, C], F16, kind="ExternalInput").ap()
    bo_d = nc.dram_tensor("bop", [128, CC], F32, kind="ExternalInput").ap()
    out_d = nc.dram_tensor("out", [BL, 128, CC * HWF], F16, kind="ExternalOutput").ap()

    x_r = x_d.rearrange("b (cc p) t h w -> b p cc (t h w)", p=128)
    wv_r = wv_d.rearrange("(cc p) d -> p cc d", p=128)
    wo_r = wo_d.rearrange("(dc p) c -> p dc c", p=128)

    with tile.TileContext(nc) as tc, ExitStack() as ctx:
        cpool = ctx.enter_context(tc.tile_pool(name="const", bufs=1))
        wpool = ctx.enter_context(tc.tile_pool(name="wts", bufs=1))
        xapool = ctx.enter_context(tc.tile_pool(name="xa", bufs=3))
        xbpool = ctx.enter_context(tc.tile_pool(name="xb", bufs=3))
        tmapool = ctx.enter_context(tc.tile_pool(name="tma", bufs=2))
        tmbpool = ctx.enter_context(tc.tile_pool(name="tmb", bufs=2))
        s1pool = ctx.enter_context(tc.tile_pool(name="s1", bufs=2))
        xwpool = ctx.enter_context(tc.tile_pool(name="xw", bufs=2))
        smpool = ctx.enter_context(tc.tile_pool(name="sm", bufs=2))
        abpool = ctx.enter_context(tc.tile_pool(name="ab", bufs=2))
        ypool = ctx.enter_context(tc.tile_pool(name="y", bufs=8))
        obpool = ctx.enter_context(tc.tile_pool(name="ob", bufs=2))
        # PSUM: [128, PSLOT] f32 tiles = exactly one 2 KB bank each.
        ps_chunk = ctx.enter_context(tc.tile_pool(name="psc", bufs=4, space="PSUM"))
        ps_small = ctx.enter_context(tc.tile_pool(name="pss", bufs=2, space="PSUM"))
        ps_proj = ctx.enter_context(tc.tile_pool(name="psp", bufs=2, space="PSUM"))

        # ---- constants (tiny DMAs on the HWDGE queue; land within ~4 us) ----
        ones7 = cpool.tile([T, 1], F32)
        nc.gpsimd.memset(ones7[:], 1.0)
        ones1 = cpool.tile([1, T], F32)
        nc.gpsimd.memset(ones1[:], 1.0)

        e_np = np.zeros((T, T * 128), dtype=np.float16)
        for t in range(T):
            e_np[t, t * 128 : (t + 1) * 128] = 1.0
        e_dram = nc.inline_tensor(e_np, name="e_ind")
        e_all = cpool.tile([T, T * 128], F16)
        nc.sync.dma_start(e_all[:], e_dram.ap())
        Es = [e_all[:, t * 128 : (t + 1) * 128] for t in range(T)]

        wqk_sb = cpool.tile([128, CC * T], F16)
        nc.sync.dma_start(wqk_sb[:], wqk_d)
        sb0 = cpool.tile([T, 1], F32)
        nc.sync.dma_start(sb0[:], sb0_d)
        boc = cpool.tile([128, CC], F32)
        nc.sync.dma_start(boc[:], bo_d)

        id_np = np.eye(128, dtype=np.float16)
        id_dram = nc.inline_tensor(id_np, name="id128")
        ident_bf = cpool.tile([128, 128], F16)
        nc.sync.dma_start(ident_bf[:], id_dram.ap())

        state = {}

        # ---- pipeline stages ----
        def load_a(b):
            xa = xapool.tile([128, CC * FA], F16, tag="xa")
            nc.gpsimd.dma_start(
                xa[:].rearrange("p (cc f) -> p cc f", f=FA),
                x_r[b][:, :, 0:FA],
            )
            state[b] = {"xa": xa}

        def load_b(b):
            xb = xbpool.tile([128, CC * FB], F16, tag="xb")
            nc.gpsimd.dma_start(
                xb[:].rearrange("p (cc f) -> p cc f", f=FB),
                x_r[b][:, :, FA:THW],
            )
            state[b]["xb"] = xb

        def load_wv():
            wv_sb = wpool.tile([128, CC * D], F16)
            nc.gpsimd.dma_start(
                wv_sb[:].rearrange("p (cc d) -> p cc d", d=D), wv_r
            )
            return wv_sb

        def load_wo():
            wo_sb = wpool.tile([128, DC * C], F16)
            nc.gpsimd.dma_start(
                wo_sb[:].rearrange("p (dc c) -> p dc c", c=C), wo_r
            )
            return wo_sb

        def scores(b):
            st = state[b]
            xa = st["xa"]
            # scores sT[t,p] from the middle frame (inside the t=0..3 group)
            stp = ps_small.tile([T, PSLOT], F32, tag="pss")
            for cc in range(CC):
                nc.tensor.matmul(
                    stp[:, 0:HWF],
                    wqk_sb[:, cc * T : (cc + 1) * T],
                    xa[:, cc * FA + TMID * HWF : cc * FA + (TMID + 1) * HWF],
                    start=(cc == 0),
                    stop=(cc == CC - 1),
                )
            s_sb = smpool.tile([T, HWF], F32, tag="s")
            nc.scalar.activation(s_sb[:], stp[:, 0:HWF], Identity, bias=sb0[:], scale=1.0)
            # softmax over t (partition dim, T=7)
            mx = smpool.tile([T, HWF], F32, tag="mx")
            nc.gpsimd.partition_all_reduce(
                mx[:], s_sb[:], channels=T, reduce_op=bass_isa.ReduceOp.max
            )
            sm = smpool.tile([T, HWF], F32, tag="smx")
            nc.vector.tensor_sub(sm[:], s_sb[:], mx[:])
            e_sb = smpool.tile([T, HWF], F32, tag="e")
            nc.scalar.activation(e_sb[:], sm[:], Exp, bias=0.0, scale=1.0)
            zp = ps_small.tile([1, PSLOT], F32, tag="pss")
            nc.tensor.matmul(zp[:, 0:HWF], ones7[:], e_sb[:], start=True, stop=True)
            rz = smpool.tile([1, HWF], F32, tag="rz")
            nc.vector.reciprocal_approx_fast(rz[:], zp[:, 0:HWF])
            rb = ps_small.tile([T, PSLOT], F32, tag="pss")
            nc.tensor.matmul(rb[:, 0:HWF], ones1[:], rz[:], start=True, stop=True)
            aT = smpool.tile([T, HWF], F16, tag="aT")
            nc.vector.tensor_mul(aT[:], e_sb[:], rb[:, 0:HWF])
            # broadcast alpha rows across the 128 c-partitions (indicator
            # matmuls, two t-rows per psum bank, strided ACT drains)
            ab = abpool.tile([128, THW], F16, tag="ab")
            for g in range(4):
                ts = [2 * g, 2 * g + 1] if g < 3 else [6]
                abp = ps_chunk.tile([128, PSLOT], F32, tag="psc", name="abp")
                for k, t in enumerate(ts):
                    nc.tensor.matmul(
                        abp[:, k * 256 : k * 256 + HWF],
                        Es[t],
                        aT[:],
                        start=True,
                        stop=True,
                    )
                if len(ts) == 2:
                    nc.scalar.copy(
                        ab[:, 2 * g * HWF : (2 * g + 2) * HWF].rearrange(
                            "p (k f) -> p k f", f=HWF
                        ),
                        abp[:].rearrange("p (k f) -> p k f", f=256)[:, :, 0:HWF],
                    )
                else:
                    nc.scalar.copy(ab[:, 6 * HWF : 7 * HWF], abp[:, 0:HWF])
            st["ab"] = ab

        def wsum_a(b):
            st = state[b]
            xa, ab = st["xa"], st["ab"]
            tma = tmapool.tile([128, CC * FA], F16, tag="tma")
            ab_a = ab[:, 0:FA].rearrange("p (r f) -> p r f", r=1).broadcast_to(
                [128, CC, FA]
            )
            nc.vector.tensor_mul(
                tma[:].rearrange("p (cc f) -> p cc f", f=FA),
                xa[:].rearrange("p (cc f) -> p cc f", f=FA),
                ab_a,
            )
            # first add level: (t0+t2, t1+t3) per chunk, all chunks in one op
            s1a = s1pool.tile([128, CC * 2 * HWF], F16, tag="s1a")
            tav = tma[:].rearrange("p (cc f) -> p cc f", f=FA)
            nc.vector.tensor_add(
                s1a[:].rearrange("p (cc f) -> p cc f", f=2 * HWF),
                tav[:, :, 0 : 2 * HWF],
                tav[:, :, 2 * HWF : 4 * HWF],
            )
            st["s1a"] = s1a

        def wsum_b(b):
            st = state[b]
            xb, ab, s1a = st["xb"], st["ab"], st["s1a"]
            tmb = tmbpool.tile([128, CC * FB], F16, tag="tmb")
            ab_b = ab[:, FA:THW].rearrange("p (r f) -> p r f", r=1).broadcast_to(
                [128, CC, FB]
            )
            nc.vector.tensor_mul(
                tmb[:].rearrange("p (cc f) -> p cc f", f=FB),
                xb[:].rearrange("p (cc f) -> p cc f", f=FB),
                ab_b,
            )
            # chunks 4..7: one more DVE add level so their PE accumulation
            # is 3-way instead of 5-way (8 fewer PE matmuls per batch)
            s1v = s1a[:].rearrange("p (cc f) -> p cc f", f=2 * HWF)
            tbv = tmb[:].rearrange("p (cc f) -> p cc f", f=FB)
            s2h = tmapool.tile([128, 4 * HWF], F16, tag="s2h", name="s2h")
            nc.vector.tensor_add(
                s2h[:].rearrange("p (cc f) -> p cc f", f=HWF),
                s1v[:, 4:CC, 0:HWF],
                s1v[:, 4:CC, HWF : 2 * HWF],
            )
            tbh = tmapool.tile([128, 4 * HWF], F16, tag="tbh", name="tbh")
            nc.vector.tensor_add(
                tbh[:].rearrange("p (cc f) -> p cc f", f=HWF),
                tbv[:, 4:CC, 0:HWF],
                tbv[:, 4:CC, HWF : 2 * HWF],
            )
            # final accumulation on PE (identity-matmul accumulate):
            # xw = s1a[0:196] + s1a[196:392] + tmb[t4] + tmb[t5] + tmb[t6]
            xw = xwpool.tile([128, CC * HWF], F16, tag="xw", name="xw")
            for pair in range(CC // 2):
                xp = ps_chunk.tile([128, PSLOT], F32, tag="psc", name="xp")
                for k in range(2):
                    cc = 2 * pair + k
                    if cc < 4:
                        movings = [
                            s1a[:, cc * 2 * HWF : cc * 2 * HWF + HWF],
                            s1a[:, cc * 2 * HWF + HWF : cc * 2 * HWF + 2 * HWF],
                            tmb[:, cc * FB : cc * FB + HWF],
                            tmb[:, cc * FB + HWF : cc * FB + 2 * HWF],
                            tmb[:, cc * FB + 2 * HWF : cc * FB + 3 * HWF],
                        ]
                    else:
                        movings = [
                            s2h[:, (cc - 4) * HWF : (cc - 3) * HWF],
                            tbh[:, (cc - 4) * HWF : (cc - 3) * HWF],
                            tmb[:, cc * FB + 2 * HWF : cc * FB + 3 * HWF],
                        ]
                    for i, mv in enumerate(movings):
                        nc.tensor.matmul(
                            xp[:, k * 256 : k * 256 + HWF],
                            ident_bf[:],
                            mv,
                            start=(i == 0),
                            stop=(i == len(movings) - 1),
                        )
                nc.scalar.copy(
                    xw[:, 2 * pair * HWF : (2 * pair + 2) * HWF].rearrange(
                        "p (k f) -> p k f", f=HWF
                    ),
                    xp[:].rearrange("p (k f) -> p k f", f=256)[:, :, 0:HWF],
                )
            st["xw"] = xw

        def proj(b):
            st = state[b]
            xw = st["xw"]
            # value projection (contract over c); bv is folded into the
            # output bias on the host, so the drain is a plain paired copy
            y_sb = ypool.tile([128, DC * HWF], F16, tag="y", name="y_sb")
            for dp in range(DC // 2):
                yp = ps_proj.tile([128, PSLOT], F32, tag="psp", name="yp")
                for k in range(2):
                    dd = 2 * dp + k
                    for cc in range(CC):
                        nc.tensor.matmul(
                            yp[:, k * 256 : k * 256 + HWF],
                            wv_sb[:, cc * D + dd * 128 : cc * D + (dd + 1) * 128],
                            xw[:, cc * HWF : (cc + 1) * HWF],
                            start=(cc == 0),
                            stop=(cc == CC - 1),
                        )
                nc.scalar.copy(
                    y_sb[:, 2 * dp * HWF : (2 * dp + 2) * HWF].rearrange(
                        "p (k f) -> p k f", f=HWF
                    ),
                    yp[:].rearrange("p (k f) -> p k f", f=256)[:, :, 0:HWF],
                )
            # output projection (contract over d), bias via ACT, merged store
            ob = obpool.tile([128, CC * HWF], F16, tag="ob", name="ob")
            for cc in range(CC):
                op = ps_proj.tile([128, PSLOT], F32, tag="psp")
                for dd in range(DC):
                    nc.tensor.matmul(
                        op[:, 0:HWF],
                        wo_sb[:, dd * C + cc * 128 : dd * C + (cc + 1) * 128],
                        y_sb[:, dd * HWF : (dd + 1) * HWF],
                        start=(dd == 0),
                        stop=(dd == DC - 1),
                    )
                nc.scalar.activation(
                    ob[:, cc * HWF : (cc + 1) * HWF],
                    op[:, 0:HWF],
                    Identity,
                    bias=boc[:, cc : cc + 1],
                    scale=1.0,
                )
            nc.sync.dma_start(out_d[b], ob[:])
            del state[b]

        # ---- software-pipelined emission ----
        # Pool/SWDGE DMA order: x0a x0b wv x1a wo x1b x2a x2b x3a x3b
        # (consts go over HWDGE and land first; x0 beats the weights)
        load_a(0)
        load_b(0)
        wv_sb = load_wv()
        scores(0)
        load_a(1)
        wo_sb = load_wo()
        load_b(1)
        wsum_a(0)
        wsum_b(0)
        scores(1)
        load_a(2)
        load_b(2)
        proj(0)
        wsum_a(1)
        wsum_b(1)
        scores(2)
        load_a(3)
        load_b(3)
        proj(1)
        wsum_a(2)
        wsum_b(2)
        scores(3)
        proj(2)
        wsum_a(3)
        wsum_b(3)
        proj(3)

    nc.compile()
    return nc


_PROG = None


def _get_prog():
    global _PROG
    if _PROG is None:
        _PROG = build_program()
    return _PROG


def _prep_inputs(inputs):
    f = lambda k: np.asarray(inputs[k], dtype=np.float32)
    x = np.ascontiguousarray(f("x_window"))
    nodes, Wq, bq, Wk, bk = f("nodes"), f("Wq"), f("bq"), f("Wk"), f("bk")
    Wv, bv, Wo, bo = f("Wv"), f("bv"), f("Wo"), f("bo")

    # host-side weight folding (batch-independent)
    k = nodes @ Wk + bk                       # [T, D]
    wqk = Wq @ k.T                            # [C, T]
    sb0 = (bq @ k.T).reshape(T, 1).astype(np.float32)          # [T, 1]
    bo_f = Wo.T @ bv + bo                     # bv folded through Wo
    wqk_p = np.ascontiguousarray(
        wqk.reshape(CC, 128, T).transpose(1, 0, 2).reshape(128, CC * T)
    ).astype(np.float16)
    bop = np.ascontiguousarray(bo_f.reshape(CC, 128).T).astype(np.float32)

    shared = {
        "wqk": wqk_p,
        "sb0": sb0,
        "Wv": Wv.astype(np.float16),
        "Wo": Wo.astype(np.float16),
        "bop": bop,
    }
    in_maps = []
    for i in range(NCORES):
        m = dict(shared)
        m["x_window"] = np.ascontiguousarray(x[i * BL : (i + 1) * BL])
        in_maps.append(m)
    return in_maps


def _unshard_out(res):
    parts = []
    for i in range(NCORES):
        ob = np.asarray(res.results[i]["out"], dtype=np.float32)  # [BL,128,CC*HWF]
        ob = ob.reshape(BL, 128, CC, HWF).transpose(0, 2, 1, 3)   # [BL,CC,128,HWF]
        parts.append(ob.reshape(BL, C, 1, H, W))
    return np.concatenate(parts, axis=0)


def kernel(**inputs):
    nc = _get_prog()
    in_maps = _prep_inputs(inputs)
    res = run_bass_kernel_spmd(nc, in_maps, core_ids=list(range(NCORES)))
    return _unshard_out(res)


# revision 25
# speedup vs baseline: 1.0336x; 1.0336x over previous
"""Trainium2 Bass kernel for LocalNodeAttentionHeadSum.

Computation (per batch b, pixel p=(h,w)):
    q[d,p]   = sum_c x[c,TMID,p] Wq[c,d] + bq[d]
    k[t,d]   = sum_c nodes[t,c] Wk[c,d] + bk[d]
    s[t,p]   = sum_d q[d,p] k[t,d];  alpha = softmax_t(s)
    y[d,p]   = sum_t alpha[t,p] * (sum_c x[c,t,p] Wv[c,d] + bv[d])
             = sum_c (sum_t alpha[t,p] x[c,t,p]) Wv[c,d] + bv[d]   (sum_t alpha = 1)
    out[c,p] = sum_d y[d,p] Wo[d,c] + bo[c]

Sharding: data-parallel over batch B=32 across 8 cores (4 batches/core).

Host-side weight folding (batch-independent, numpy): the query projection
and key path fold into  Wqk = Wq @ (nodes @ Wk + bk).T  ([C, T]) and
sb0[t] = bq . k[t], so the device never sees Wq/Wk/nodes/bq/bk; scores
are 8 thin [128c x 7] matmuls per batch.  Wv / Wo ship as fp16.

DMA regime (the kernel is HBM-bound): x streams through fp32->fp16
casting SWDGE DMAs (gpsimd), which move half the bytes into SBUF that an
fp32 load would.  Each batch is two DMAs - the t=0..3 frames (includes
the middle frame, unlocking scores/softmax early) and the t=4..6 frames -
with every descriptor a contiguous >=1 KB run; the x stream is issued
ahead of the weight loads so batch 0 lands as early as possible.  The
output is stored as fp16 with per-partition-contiguous 3 KB descriptors
and widened to fp32 on the host.

fp16 (not bf16) is deliberate: same DMA/DVE/PE cost in every model path,
but 10 mantissa bits keep the score error ~8x smaller, which the softmax
then cannot amplify into the output.

Engine split per batch: PE does scores + alpha row-broadcast + the final
5-way temporal accumulation + both projections; DVE does the alpha
multiplies and the first add level; ACT applies biases, exp and
PSUM->SBUF moves; GPSIMD does DMA descriptor gen and the per-pixel
partition max for softmax.
"""

import sys

for _p in ("/opt/trn_rl_repo",):
    if _p not in sys.path:
        sys.path.insert(0, _p)

from contextlib import ExitStack

import numpy as np

import concourse.bass as bass
import concourse.tile as tile
from concourse import bacc, mybir, bass_isa
from concourse.bass_utils import run_bass_kernel_spmd

F32 = mybir.dt.float32
F16 = mybir.dt.float16

# Problem shapes (hardcoded per contract)
B, C, T, H, W = 32, 1024, 7, 14, 14
D = 512
NCORES = 8
BL = B // NCORES          # 4 batches per core
HWF = H * W               # 196
THW = T * HWF             # 1372
CC = C // 128             # 8 chunks over input channels
DC = D // 128             # 4 chunks over inter channels
TMID = T // 2             # 3 (middle frame)
TA = 4                    # first t-group (t=0..3, includes TMID)
TB = T - TA               # second t-group (t=4..6)
FA = TA * HWF             # 784
FB = TB * HWF             # 588
PSLOT = 512               # full psum bank (f32 elems) per small tile

Exp = mybir.ActivationFunctionType.Exp
Identity = mybir.ActivationFunctionType.Identity


def build_program():
    nc = bacc.Bacc("TRN2", target_bir_lowering=False, debug=False)

    x_d = nc.dram_tensor("x_window", [BL, C, T, H, W], F32, kind="ExternalInput").ap()
    wqk_d = nc.dram_tensor("wqk", [128, CC * T], F16, kind="ExternalInput").ap()
    sb0_d = nc.dram_tensor("sb0", [T, 1], F32, kind="ExternalInput").ap()
    wv_d = nc.dram_tensor("Wv", [C, D], F16, kind="ExternalInput").ap()
    wo_d = nc.dram_tensor("Wo", [D, C], F16, kind="ExternalInput").ap()
    bo_d = nc.dram_tensor("bop", [128, CC], F32, kind="ExternalInput").ap()
    out_d = nc.dram_tensor("out", [BL, 128, CC * HWF], F16, kind="ExternalOutput").ap()

    x_r = x_d.rearrange("b (cc p) t h w -> b p cc (t h w)", p=128)
    wv_r = wv_d.rearrange("(cc p) d -> p cc d", p=128)
    wo_r = wo_d.rearrange("(dc p) c -> p dc c", p=128)

    with tile.TileContext(nc) as tc, ExitStack() as ctx:
        cpool = ctx.enter_context(tc.tile_pool(name="const", bufs=1))
        wpool = ctx.enter_context(tc.tile_pool(name="wts", bufs=1))
        xapool = ctx.enter_context(tc.tile_pool(name="xa", bufs=3))
        xbpool = ctx.enter_context(tc.tile_pool(name="xb", bufs=3))
        tmapool = ctx.enter_context(tc.tile_pool(name="tma", bufs=2))
        tmbpool = ctx.enter_context(tc.tile_pool(name="tmb", bufs=2))
        s1pool = ctx.enter_context(tc.tile_pool(name="s1", bufs=2))
        xwpool = ctx.enter_context(tc.tile_pool(name="xw", bufs=2))
        smpool = ctx.enter_context(tc.tile_pool(name="sm", bufs=2))
        abpool = ctx.enter_context(tc.tile_pool(name="ab", bufs=2))
        ypool = ctx.enter_context(tc.tile_pool(name="y", bufs=8))
        obpool = ctx.enter_context(tc.tile_pool(name="ob", bufs=2))
        # PSUM: [128, PSLOT] f32 tiles = exactly one 2 KB bank each.
        ps_chunk = ctx.enter_context(tc.tile_pool(name="psc", bufs=4, space="PSUM"))
        ps_small = ctx.enter_context(tc.tile_pool(name="pss", bufs=2, space="PSUM"))
        ps_proj = ctx.enter_context(tc.tile_pool(name="psp", bufs=2, space="PSUM"))

        # ---- constants (tiny DMAs on the HWDGE queue; land within ~4 us) ----
        ones7 = cpool.tile([T, 1], F32)
        nc.gpsimd.memset(ones7[:], 1.0)
        ones1 = cpool.tile([1, T], F32)
        nc.gpsimd.memset(ones1[:], 1.0)

        e_np = np.zeros((T, T * 128), dtype=np.float16)
        for t in range(T):
            e_np[t, t * 128 : (t + 1) * 128] = 1.0
        e_dram = nc.inline_tensor(e_np, name="e_ind")
        e_all = cpool.tile([T, T * 128], F16)
        nc.sync.dma_start(e_all[:], e_dram.ap())
        Es = [e_all[:, t * 128 : (t + 1) * 128] for t in range(T)]

        wqk_sb = cpool.tile([128, CC * T], F16)
        nc.sync.dma_start(wqk_sb[:], wqk_d)
        sb0 = cpool.tile([T, 1], F32)
        nc.sync.dma_start(sb0[:], sb0_d)
        boc = cpool.tile([128, CC], F32)
        nc.sync.dma_start(boc[:], bo_d)

        id_np = np.eye(128, dtype=np.float16)
        id_dram = nc.inline_tensor(id_np, name="id128")
        ident_bf = cpool.tile([128, 128], F16)
        nc.sync.dma_start(ident_bf[:], id_dram.ap())

        state = {}

        # ---- pipeline stages ----
        def load_a(b):
            xa = xapool.tile([128, CC * FA], F16, tag="xa")
            nc.gpsimd.dma_start(
                xa[:].rearrange("p (cc f) -> p cc f", f=FA),
                x_r[b][:, :, 0:FA],
            )
            state[b] = {"xa": xa}

        def load_b(b):
            xb = xbpool.tile([128, CC * FB], F16, tag="xb")
            nc.gpsimd.dma_start(
                xb[:].rearrange("p (cc f) -> p cc f", f=FB),
                x_r[b][:, :, FA:THW],
            )
            state[b]["xb"] = xb

        def load_wv():
            wv_sb = wpool.tile([128, CC * D], F16)
            nc.gpsimd.dma_start(
                wv_sb[:].rearrange("p (cc d) -> p cc d", d=D), wv_r
            )
            return wv_sb

        def load_wo():
            wo_sb = wpool.tile([128, DC * C], F16)
            nc.gpsimd.dma_start(
                wo_sb[:].rearrange("p (dc c) -> p dc c", c=C), wo_r
            )
            return wo_sb

        def scores(b):
            st = state[b]
            xa = st["xa"]
            # scores sT[t,p] from the middle frame (inside the t=0..3 group)
            stp = ps_small.tile([T, PSLOT], F32, tag="pss")
            for cc in range(CC):
                nc.tensor.matmul(
                    stp[:, 0:HWF],
                    wqk_sb[:, cc * T : (cc + 1) * T],
                    xa[:, cc * FA + TMID * HWF : cc * FA + (TMID + 1) * HWF],
                    start=(cc == 0),
                    stop=(cc == CC - 1),
                )
            s_sb = smpool.tile([T, HWF], F32, tag="s")
            nc.scalar.activation(s_sb[:], stp[:, 0:HWF], Identity, bias=sb0[:], scale=1.0)
            # softmax over t (partition dim, T=7)
            mx = smpool.tile([T, HWF], F32, tag="mx")
            nc.gpsimd.partition_all_reduce(
                mx[:], s_sb[:], channels=T, reduce_op=bass_isa.ReduceOp.max
            )
            sm = smpool.tile([T, HWF], F32, tag="smx")
            nc.vector.tensor_sub(sm[:], s_sb[:], mx[:])
            e_sb = smpool.tile([T, HWF], F32, tag="e")
            nc.scalar.activation(e_sb[:], sm[:], Exp, bias=0.0, scale=1.0)
            zp = ps_small.tile([1, PSLOT], F32, tag="pss")
            nc.tensor.matmul(zp[:, 0:HWF], ones7[:], e_sb[:], start=True, stop=True)
            rz = smpool.tile([1, HWF], F32, tag="rz")
            nc.vector.reciprocal_approx_fast(rz[:], zp[:, 0:HWF])
            rb = ps_small.tile([T, PSLOT], F32, tag="pss")
            nc.tensor.matmul(rb[:, 0:HWF], ones1[:], rz[:], start=True, stop=True)
            aT = smpool.tile([T, HWF], F16, tag="aT")
            nc.vector.tensor_mul(aT[:], e_sb[:], rb[:, 0:HWF])
            # broadcast alpha rows across the 128 c-partitions (indicator
            # matmuls, two t-rows per psum bank, strided ACT drains)
            ab = abpool.tile([128, THW], F16, tag="ab")
            for g in range(4):
                ts = [2 * g, 2 * g + 1] if g < 3 else [6]
                abp = ps_chunk.tile([128, PSLOT], F32, tag="psc", name="abp")
                for k, t in enumerate(ts):
                    nc.tensor.matmul(
                        abp[:, k * 256 : k * 256 + HWF],
                        Es[t],
                        aT[:],
                        start=True,
                        stop=True,
                    )
                if len(ts) == 2:
                    nc.scalar.copy(
                        ab[:, 2 * g * HWF : (2 * g + 2) * HWF].rearrange(
                            "p (k f) -> p k f", f=HWF
                        ),
                        abp[:].rearrange("p (k f) -> p k f", f=256)[:, :, 0:HWF],
                    )
                else:
                    nc.scalar.copy(ab[:, 6 * HWF : 7 * HWF], abp[:, 0:HWF])
            st["ab"] = ab

        def wsum_a(b):
            st = state[b]
            xa, ab = st["xa"], st["ab"]
            tma = tmapool.tile([128, CC * FA], F16, tag="tma")
            for cc in range(CC):
                nc.vector.tensor_mul(
                    tma[:, cc * FA : (cc + 1) * FA],
                    xa[:, cc * FA : (cc + 1) * FA],
                    ab[:, 0:FA],
                )
            # first add level: (t0+t2, t1+t3) per chunk, all chunks in one op
            s1a = s1pool.tile([128, CC * 2 * HWF], F16, tag="s1a")
            tav = tma[:].rearrange("p (cc f) -> p cc f", f=FA)
            nc.vector.tensor_add(
                s1a[:].rearrange("p (cc f) -> p cc f", f=2 * HWF),
                tav[:, :, 0 : 2 * HWF],
                tav[:, :, 2 * HWF : 4 * HWF],
            )
            st["s1a"] = s1a

        def wsum_b(b):
            st = state[b]
            xb, ab, s1a = st["xb"], st["ab"], st["s1a"]
            tmb = tmbpool.tile([128, CC * FB], F16, tag="tmb")
            for cc in range(CC):
                nc.vector.tensor_mul(
                    tmb[:, cc * FB : (cc + 1) * FB],
                    xb[:, cc * FB : (cc + 1) * FB],
                    ab[:, FA:THW],
                )
            # final accumulation on PE (identity-matmul accumulate):
            # xw = s1a[0:196] + s1a[196:392] + tmb[t4] + tmb[t5] + tmb[t6]
            xw = xwpool.tile([128, CC * HWF], F16, tag="xw", name="xw")
            for pair in range(CC // 2):
                xp = ps_chunk.tile([128, PSLOT], F32, tag="psc", name="xp")
                for k in range(2):
                    cc = 2 * pair + k
                    movings = [
                        s1a[:, cc * 2 * HWF : cc * 2 * HWF + HWF],
                        s1a[:, cc * 2 * HWF + HWF : cc * 2 * HWF + 2 * HWF],
                        tmb[:, cc * FB : cc * FB + HWF],
                        tmb[:, cc * FB + HWF : cc * FB + 2 * HWF],
                        tmb[:, cc * FB + 2 * HWF : cc * FB + 3 * HWF],
                    ]
                    for i, mv in enumerate(movings):
                        nc.tensor.matmul(
                            xp[:, k * 256 : k * 256 + HWF],
                            ident_bf[:],
                            mv,
                            start=(i == 0),
                            stop=(i == len(movings) - 1),
                        )
                nc.scalar.copy(
                    xw[:, 2 * pair * HWF : (2 * pair + 2) * HWF].rearrange(
                        "p (k f) -> p k f", f=HWF
                    ),
                    xp[:].rearrange("p (k f) -> p k f", f=256)[:, :, 0:HWF],
                )
            st["xw"] = xw

        def proj(b):
            st = state[b]
            xw = st["xw"]
            # value projection (contract over c); bv is folded into the
            # output bias on the host, so the drain is a plain paired copy
            y_sb = ypool.tile([128, DC * HWF], F16, tag="y", name="y_sb")
            for dp in range(DC // 2):
                yp = ps_proj.tile([128, PSLOT], F32, tag="psp", name="yp")
                for k in range(2):
                    dd = 2 * dp + k
                    for cc in range(CC):
                        nc.tensor.matmul(
                            yp[:, k * 256 : k * 256 + HWF],
                            wv_sb[:, cc * D + dd * 128 : cc * D + (dd + 1) * 128],
                            xw[:, cc * HWF : (cc + 1) * HWF],
                            start=(cc == 0),
                            stop=(cc == CC - 1),
                        )
                nc.scalar.copy(
                    y_sb[:, 2 * dp * HWF : (2 * dp + 2) * HWF].rearrange(
                        "p (k f) -> p k f", f=HWF
                    ),
                    yp[:].rearrange("p (k f) -> p k f", f=256)[:, :, 0:HWF],
                )
            # output projection (contract over d), bias via ACT, merged store
            ob = obpool.tile([128, CC * HWF], F16, tag="ob", name="ob")
            for cc in range(CC):
                op = ps_proj.tile([128, PSLOT], F32, tag="psp")
                for dd in range(DC):
                    nc.tensor.matmul(
                        op[:, 0:HWF],
                        wo_sb[:, dd * C + cc * 128 : dd * C + (cc + 1) * 128],
                        y_sb[:, dd * HWF : (dd + 1) * HWF],
                        start=(dd == 0),
                        stop=(dd == DC - 1),
                    )
                nc.scalar.activation(
                    ob[:, cc * HWF : (cc + 1) * HWF],
                    op[:, 0:HWF],
                    Identity,
                    bias=boc[:, cc : cc + 1],
                    scale=1.0,
                )
            nc.sync.dma_start(out_d[b], ob[:])
            del state[b]

        # ---- software-pipelined emission ----
        # Pool/SWDGE DMA order: x0a x0b wv x1a wo x1b x2a x2b x3a x3b
        # (consts go over HWDGE and land first; x0 beats the weights)
        load_a(0)
        load_b(0)
        wv_sb = load_wv()
        scores(0)
        load_a(1)
        wo_sb = load_wo()
        load_b(1)
        wsum_a(0)
        wsum_b(0)
        scores(1)
        load_a(2)
        load_b(2)
        proj(0)
        wsum_a(1)
        wsum_b(1)
        scores(2)
        load_a(3)
        load_b(3)
        proj(1)
        wsum_a(2)
        wsum_b(2)
        scores(3)
        proj(2)
        wsum_a(3)
        wsum_b(3)
        proj(3)

    nc.compile()
    return nc


_PROG = None


def _get_prog():
    global _PROG
    if _PROG is None:
        _PROG = build_program()
    return _PROG


def _prep_inputs(inputs):
    f = lambda k: np.asarray(inputs[k], dtype=np.float32)
    x = np.ascontiguousarray(f("x_window"))
    nodes, Wq, bq, Wk, bk = f("nodes"), f("Wq"), f("bq"), f("Wk"), f("bk")
    Wv, bv, Wo, bo = f("Wv"), f("bv"), f("Wo"), f("bo")

    # host-side weight folding (batch-independent)
    k = nodes @ Wk + bk                       # [T, D]
    wqk = Wq @ k.T                            # [C, T]
    sb0 = (bq @ k.T).reshape(T, 1).astype(np.float32)          # [T, 1]
    bo_f = Wo.T @ bv + bo                     # bv folded through Wo
    wqk_p = np.ascontiguousarray(
        wqk.reshape(CC, 128, T).transpose(1, 0, 2).reshape(128, CC * T)
    ).astype(np.float16)
    bop = np.ascontiguousarray(bo_f.reshape(CC, 128).T).astype(np.float32)

    shared = {
        "wqk": wqk_p,
        "sb0": sb0,
        "Wv": Wv.astype(np.float16),
        "Wo": Wo.astype(np.float16),
        "bop": bop,
    }
    in_maps = []
    for i in range(NCORES):
        m = dict(shared)
        m["x_window"] = np.ascontiguousarray(x[i * BL : (i + 1) * BL])
        in_maps.append(m)
    return in_maps


def _unshard_out(res):
    parts = []
    for i in range(NCORES):
        ob = np.asarray(res.results[i]["out"], dtype=np.float32)  # [BL,128,CC*HWF]
        ob = ob.reshape(BL, 128, CC, HWF).transpose(0, 2, 1, 3)   # [BL,CC,128,HWF]
        parts.append(ob.reshape(BL, C, 1, H, W))
    return np.concatenate(parts, axis=0)


def kernel(**inputs):
    nc = _get_prog()
    in_maps = _prep_inputs(inputs)
    res = run_bass_kernel_spmd(nc, in_maps, core_ids=list(range(NCORES)))
    return _unshard_out(res)
